# revision 66
# baseline (speedup 1.0000x reference)
"""TRN2 Bass kernel for nn_HeartDisDet: embed-lookup + 44->256->128->2 MLP.

Strategy (8-way batch data-parallel, B=524288 -> 65536/core):
  - The 7 categorical embedding lookups are folded into the first matmul
    as exact polynomials in the raw category value (19 logical features;
    constant row carries b1). Host encodes the layout.
  - L1 runs as fp8e4m3 DoubleRow matmuls (0.5 cycles/row): the 19
    features and the folded weights are each split hi+lo into fp8 and
    laid out as K=50 rows (25 partitions x 2 k-tiles), making the fp8
    quantization error second-order (~1e-3) while halving PE cost.
  - h1 = tanh(z1) on ACT (bf16 out); L2 = W2^T h1 in bf16 (2 matmuls,
    K=256); L3 in bf16 with partition-packed PSUM (4 chunks/bank via
    tile_position), drained by DVE copies and repacked by DMAs.
  - h2 = tanh(z2): ~99/128 of chunks evaluate on the DVE via two
    PSUM-direct custom ops (1-Newton bitwise-seed reciprocal:
    OpA: yh ~ k0/(x^2+c); OpB: t = x(1 + a x^2) * yh; max |err| ~6e-3
    on |x|<=3.6), the rest on ACT with the bias fused.  The DVE path
    requires b2 == 0 (true for this problem's inputs); a nonzero b2
    falls back to all-ACT h2.  GPSIMD cannot read PSUM on real HW
    (and AP-scalars / 1-D Src1 in custom ops crash the device), so the
    Pool engine stays idle and the balance is ACT vs DVE.
  - Feature-major layout: activations are [hidden, samples] tiles.
    Emission is software-pipelined (L1/h1 | L2/h2 one chunk behind |
    L3/drain CDEPTH chunks behind) so no engine's in-order queue
    contains a same-chunk cross-engine round trip.
"""

import os
import re
import sys

if "/opt/trn_rl_repo" not in sys.path:
    sys.path.insert(0, "/opt/trn_rl_repo")
os.environ.setdefault("BASS_NEVER_TRACE", "1")

from contextlib import ExitStack

import ml_dtypes
import numpy as np

import concourse.bass as bass
import concourse.dve_ops as dvo
import concourse.mybir as mybir
import concourse.tile as tile
from concourse import bacc
from concourse.bass_utils import run_bass_kernel_spmd
from concourse.dve_spec import AluOp, Bin, One, Spec, Src0, Src1, C0, C1, C2, sq

N_CORES = 8
B_TOTAL = 524288
B_CORE = B_TOTAL // N_CORES   # 65536
CHUNK = 512                   # samples per matmul (PSUM bank = 512 fp32)
GROUP = 4096                  # samples per input DMA
N_GROUPS = B_CORE // GROUP    # 16
CHUNKS_PER_GROUP = GROUP // CHUNK    # 8
ROUND = 16384                 # samples per output round (32 chunks)
CHUNKS_PER_ROUND = ROUND // CHUNK    # 32
N_ROUNDS = B_CORE // ROUND    # 4
OCOL = ROUND // 64            # 256 output columns in the repack layout
K_P = 25                      # L1 DoubleRow contraction partitions
# k-tile rows: [19 features | 6 con_x-lo residual features] x {W_hi, W_lo}

# pipeline depth of the L3/drain stage behind the L1/h1 stage
CDEPTH = 4
# of every 128 chunks, this many h2 evaluations go to the DVE custom-op
# pair (PSUM-direct; requires b2 == 0); the rest stay on ACT.  GPSIMD
# cannot touch PSUM on real HW, so the Pool engine is unusable here and
# the balance is ACT (h1 + some h2) vs DVE (most h2).
NDVE128 = 101
# h2 chunks at the very end forced to ACT (tail convergence)
NTAIL2 = 4

TRACE = False
LAST = {}

# tanh(x) ~ x(1 + a s) * (k0/(s + c)), s = x^2, with the k0/(s+c) factor
# computed as a scaled 1-Newton reciprocal from a bitwise-NOT seed
# (constants tuned end-to-end in fp16/bf16, max err ~6.1e-3 on |x|<=3.6).
TA_A = 0.04404854372589698
TA_C = 2.844656450782092
TA_S1 = -0.3924570858306584
TA_M = 3.3526810765647515


def _register_op(op: "dvo.DveOp") -> "dvo.DveOp":
    """Register a custom DVE op, pinning uops_sha by trial-compiling (the
    sha mismatch error carries the computed value). Idempotent."""
    if op.name in dvo._SUB_OPCODE_FOR_NAME:
        return next(o for o in dvo.OPS if o.name == op.name)
    dvo._SUB_OPCODE_FOR_NAME[op.name] = dvo._CUSTOM_DVE_ROW_BASE + len(dvo.OPS)
    shas = {}
    for ver in ("v3", "v4"):
        try:
            op.compile(ver)
            shas[ver] = op.uops_sha.get(ver)
        except ValueError as e:
            m = re.search(rf"\({ver}: ([0-9a-f]+) ", str(e))
            if not m:
                raise
            shas[ver] = m.group(1)
    pinned = dvo.DveOp(name=op.name, spec=op.spec, subdim=op.subdim,
                       uops_sha=shas, perf_en=op.perf_en)
    dvo.OPS.append(pinned)
    dvo.CUSTOM_DVE_SPECS[pinned.name] = pinned.spec
    return pinned


def _ref_tanh_recip(in0, in1, c0, c1, c2):
    x = in0.astype(np.float32)
    d = x * x + np.float32(c0)
    nd = (~d.view(np.int32)).view(np.float32)
    y0 = nd * np.float32(c1)
    return y0 * (np.float32(c2) - d * y0)


_d = sq(Src0) + C0
_nd = Bin(AluOp.BITWISE_NOT, _d, _d)
_y0 = _nd * C1
TANH_RECIP_ANT = _register_op(dvo.DveOp(
    "TANH_RECIP_ANT",
    Spec(body=_y0 * (C2 - _d * _y0), reference=_ref_tanh_recip),
    subdim=False,
    uops_sha={},
))

TANH_NUM_ANT = _register_op(dvo.DveOp(
    "TANH_NUM_ANT",
    Spec(
        body=((sq(Src0) * C0 + One) * Src0) * Src1,
        reference=lambda in0, in1, c0, c1, c2: (
            (in0.astype(np.float32) * in0 * np.float32(c0) + np.float32(1.0))
            * in0 * in1).astype(np.float32),
    ),
    subdim=False,
    uops_sha={},
))


_CACHE = {}


def _h2_offload(c):
    """Evenly-spread Bresenham pattern: NDVE128 of every 128 h2 chunks -> DVE.
    The last NTAIL2 chunks stay on ACT so both engines drain together."""
    if c >= N_ROUNDS * CHUNKS_PER_ROUND - NTAIL2:
        return False
    cp = (c + 1) % 128
    return cp * NDVE128 // 128 != (cp + 1) * NDVE128 // 128


def _fold_weights(emb2, emb3, emb4, W1, b1):
    """Fold embedding tables + b1 into Wt [19, 256] (fp64 math)."""
    W1 = np.asarray(W1, dtype=np.float64)
    Wt = np.zeros((19, 256), dtype=np.float64)
    bt = np.asarray(b1, dtype=np.float64).copy()
    Wt[0:6] = W1[38:44]                          # con_x
    for i in range(3):                           # vocab-2: T0 + c*(T1-T0)
        T = np.asarray(emb2, np.float64)[i] @ W1[4 * i:4 * i + 4]
        Wt[6 + i] = T[1] - T[0]
        bt += T[0]
    for i in range(3):                           # vocab-3: quadratic fit
        S = np.asarray(emb3, np.float64)[i] @ W1[12 + 6 * i:18 + 6 * i]
        Wt[9 + i] = -1.5 * S[0] + 2.0 * S[1] - 0.5 * S[2]
        Wt[14 + i] = 0.5 * S[0] - S[1] + 0.5 * S[2]
        bt += S[0]
    # vocab-4 on basis {1, c, c^2, relu(c-2)}; exact on c in {0,1,2,3}
    S = np.asarray(emb4, np.float64) @ W1[30:38]
    V = np.array([[1, 0, 0, 0], [1, 1, 1, 0], [1, 2, 4, 0], [1, 3, 9, 1]],
                 dtype=np.float64)
    A = np.linalg.solve(V, S)
    Wt[12] = A[1]
    Wt[17] = A[2]
    Wt[18] = A[3]
    bt += A[0]
    Wt[13] = bt                                  # ones row carries bias
    return Wt


def _build_nc(dve_h2=True):
    f16 = mybir.dt.float16
    bf16 = mybir.dt.bfloat16
    f32 = mybir.dt.float32
    f8 = mybir.dt.float8e4
    DR = mybir.MatmulPerfMode.DoubleRow
    nc = bacc.Bacc(None, target_bir_lowering=False)
    # xdr carries w1 in its first 256 columns so one DMA lands both the
    # L1 weights and the first samples (cuts the fill latency chain)
    x_d = nc.dram_tensor("xdr", [K_P, 2, 256 + B_CORE], f8,
                         kind="ExternalInput")
    w2_d = nc.dram_tensor("w2", [2, 128, 128], bf16, kind="ExternalInput")
    w3_d = nc.dram_tensor("w3", [128, 2], bf16, kind="ExternalInput")
    bb_d = nc.dram_tensor("bb", [128, 2], f32, kind="ExternalInput")
    o_d = nc.dram_tensor("out", [N_ROUNDS, 2, 64, OCOL], f32,
                         kind="ExternalOutput")

    with nc.allow_low_precision(reason="fp8 DR L1, fp16 tanh, bf16 L2/L3"), \
            tile.TileContext(nc) as tc, ExitStack() as ctx:
        singles = ctx.enter_context(tc.tile_pool(name="singles", bufs=1))
        xpool = ctx.enter_context(tc.tile_pool(name="xg", bufs=4))
        h1pool = ctx.enter_context(tc.tile_pool(name="h1", bufs=5))
        h2pool = ctx.enter_context(tc.tile_pool(name="h2", bufs=8))
        rtpool = ctx.enter_context(tc.tile_pool(name="rt", bufs=3))
        opool = ctx.enter_context(tc.tile_pool(name="osb", bufs=2))
        stpool = ctx.enter_context(tc.tile_pool(name="stage", bufs=3))
        p1pool = ctx.enter_context(tc.tile_pool(name="p1", bufs=2, space="PSUM"))
        p2pool = ctx.enter_context(tc.tile_pool(name="p2", bufs=3, space="PSUM"))
        p3pool = ctx.enter_context(tc.tile_pool(name="p3", bufs=1, space="PSUM"))

        w2 = singles.tile([128, 2, 128], bf16)
        w3 = singles.tile([128, 2], bf16)
        bb = singles.tile([128, 2], f32)
        b2 = bb[:, 0:1]
        b3t = bb[:, 1:2]
        # tiny dummy matmul on a zeroed tile to start the PE p-state ramp
        # immediately, so the first real L1 matmuls run at full clock
        wsrc = singles.tile([8, 8], bf16)
        nc.gpsimd.memset(wsrc, 0.0)
        warm = p3pool.tile([8, 8], mybir.dt.float32, tag="p3")
        nc.tensor.matmul(warm, wsrc, wsrc, start=True, stop=True)

        tanh = mybir.ActivationFunctionType.Tanh
        sigm = mybir.ActivationFunctionType.Sigmoid


        n_chunks = N_ROUNDS * CHUNKS_PER_ROUND
        h1_of = {}
        h2_of = {}
        p3_of = {}
        osb_of = {}
        st4_of = {}

        # software-pipelined emission: per step s, stage A handles chunk s
        # (load/L1/h1 on ACT or the DVE custom-op pair), stage B chunk s-1
        # (L2 + ACT tanh h2), stage C chunk s-CDEPTH (L3/drain/output).
        for step in range(n_chunks + CDEPTH):
            # ---- stage A: chunk s ----
            if step < n_chunks:
                c = step
                g, cg = divmod(c, CHUNKS_PER_GROUP)
                if cg == 0:
                    if g == 0:
                        # group-0 tile leads with the 256 w1 columns
                        xg = xpool.tile([K_P, 2, 256 + GROUP], f8, tag="xg0")
                        w1 = xg[:, :, 0:256]
                        h = GROUP // 4
                        for q in range(4):
                            o0 = 256 + q * h if q else 0
                            o1 = 256 + (q + 1) * h
                            nc.sync.dma_start(
                                out=xg[:, :, o0:o1],
                                in_=x_d[:, :, o0:o1])
                            if q == 0:
                                nc.sync.dma_start(
                                    out=w2,
                                    in_=w2_d.rearrange("t p m -> p t m"))
                                nc.sync.dma_start(out=bb, in_=bb_d[:, :])
                        goff = 256
                    else:
                        xg = xpool.tile([K_P, 2, GROUP], f8, tag="xg")
                        nc.sync.dma_start(
                            out=xg,
                            in_=x_d[:, :, 256 + g * GROUP:256 + (g + 1) * GROUP])
                        goff = 0
                rhs = xg[:, :, goff + cg * CHUNK:goff + (cg + 1) * CHUNK]
                p1 = p1pool.tile([128, 2 * CHUNK], f32)
                nc.tensor.matmul(p1[:, 0:CHUNK], w1[:, :, 0:128], rhs,
                                 start=True, stop=True, perf_mode=DR)
                nc.tensor.matmul(p1[:, CHUNK:2 * CHUNK], w1[:, :, 128:256],
                                 rhs, start=True, stop=True, perf_mode=DR)
                h1 = h1pool.tile([128, 2 * CHUNK], bf16, tag="h1")
                nc.scalar.activation(h1, p1, tanh)
                h1_of[c] = h1
                if c == 0:
                    nc.sync.dma_start(out=w3, in_=w3_d[:, :])
            # ---- stage B: chunk s-1 ----
            if 0 <= step - 1 < n_chunks:
                c = step - 1
                h1 = h1_of.pop(c)
                p2 = p2pool.tile([128, CHUNK], f32)
                nc.tensor.matmul(p2, w2[:, 0, :], h1[:, 0:CHUNK],
                                 start=True, stop=False)
                nc.tensor.matmul(p2, w2[:, 1, :], h1[:, CHUNK:2 * CHUNK],
                                 start=False, stop=True)
                h2 = h2pool.tile([128, CHUNK], bf16, tag="h2")
                if dve_h2 and _h2_offload(c):
                    # PSUM-direct custom-op pair on the DVE (b2 == 0 here;
                    # AP scalars / 1-D Src1 crash the device, so a nonzero
                    # bias falls back to the ACT path below)
                    rt = rtpool.tile([128, CHUNK], f16, tag="rt")
                    nc.vector._custom_dve(TANH_RECIP_ANT, out=rt, in0=p2,
                                          s0=TA_C, s1=TA_S1, imm2=TA_M)
                    nc.vector._custom_dve(TANH_NUM_ANT, out=h2, in0=p2,
                                          in1=rt, s0=TA_A, s1=0.0)
                else:
                    nc.scalar.activation(h2, p2, tanh, bias=b2)
                h2_of[c] = h2
            # ---- stage C: chunk s-CDEPTH ----
            if 0 <= step - CDEPTH < n_chunks:
                c = step - CDEPTH
                rd, cc = divmod(c, CHUNKS_PER_ROUND)
                blk, i = divmod(cc, 4)           # z3 bank block / slot
                if i == 0:
                    p3_t = p3pool.tile([128, CHUNK], f32, tag="p3")
                    p3_of[blk] = p3_t
                p3 = p3_of[blk]
                h2v = h2_of.pop(c)
                # partition-packed L3: chunk slot i -> psum partitions 32i..+1
                nc.tensor.matmul(p3[32 * i:32 * i + 2, :], w3, h2v,
                                 start=True, stop=True,
                                 tile_position=(0, 32 * i))
                if i == 3:
                    # one DVE copy drains 4 chunks' [2, 512] z3 outputs
                    # (neither DMA nor GPSIMD can read PSUM), then the
                    # repack DMAs write z3 straight to DRAM; the host
                    # applies the final sigmoid in fp64.
                    stage = stpool.tile([128, CHUNK], f32, tag="stage")
                    nc.vector.tensor_copy(stage[0:98, :], p3[0:98, :])
                    # o_d[rd, o, 8b+2i+j, n] = z3(4b+i)[o, 256j+n]
                    for o in range(2):
                        nc.sync.dma_start(
                            out=o_d[rd, o, 8 * blk:8 * blk + 8, :],
                            in_=stage[o:o + 97:32, :]
                            .rearrange("p (j n) -> p j n", n=OCOL))
    nc.finalize()
    return nc


def kernel(con_x, cat_2, cat_3, cat_4, emb2, emb3, emb4,
           W1, b1, W2, b2, W3, b3):
    f8np = ml_dtypes.float8_e4m3
    bf16np = ml_dtypes.bfloat16
    B = con_x.shape[0]
    assert B == B_TOTAL

    Wt = _fold_weights(emb2, emb3, emb4, W1, b1)     # [19, 256] fp64

    # --- L1 DoubleRow operands: hi/lo fp8 split of features and weights ---
    # x rows (identical for both k-tiles): [19 features | 6 con_x lo-residual]
    xrow = np.empty((K_P, B), dtype=np.float32)
    cx = np.asarray(con_x, dtype=np.float32).T
    cxh = cx.astype(f8np).astype(np.float32)
    c3 = np.asarray(cat_3, dtype=np.float32).T
    c4 = np.asarray(cat_4, dtype=np.float32).T
    xrow[0:6] = cxh
    xrow[6:9] = np.asarray(cat_2, dtype=np.float32).T
    xrow[9:12] = c3
    xrow[12] = c4[0]
    xrow[13] = 1.0
    xrow[14:17] = c3 * c3
    xrow[17] = c4[0] * c4[0]
    xrow[18] = np.maximum(c4[0] - 2.0, 0.0)
    xrow[19:25] = cx - cxh                           # con_x lo residual
    x8 = xrow.astype(f8np)                           # [25, B]
    xdr = np.ascontiguousarray(
        np.broadcast_to(x8[:, None, :], (K_P, 2, B)))

    # weight k-tiles: hi rows, then lo rows (same x both times)
    Whi = Wt.astype(np.float32).astype(f8np).astype(np.float64)
    Wlo = (Wt - Whi).astype(np.float32).astype(f8np).astype(np.float64)
    # ktile0 = W_hi for all 25 x-rows, ktile1 = W_lo: together they form
    # (xh+xl)(Whi+Wlo) exactly, so the fp8 error is second-order.
    w1dr = np.zeros((K_P, 2, 256), dtype=np.float64)
    w1dr[0:19, 0] = Whi
    w1dr[19:25, 0] = Whi[0:6]
    w1dr[0:19, 1] = Wlo
    w1dr[19:25, 1] = Wlo[0:6]
    w1_np = w1dr.astype(np.float32).astype(f8np)          # [25, 2, 256]

    w2_np = np.ascontiguousarray(
        np.asarray(W2, dtype=np.float32).reshape(2, 128, 128).astype(bf16np))
    w3_np = np.ascontiguousarray(
        np.asarray(W3, dtype=np.float32).astype(bf16np))
    bb_np = np.empty((128, 2), dtype=np.float32)
    bb_np[:, 0] = np.asarray(b2, dtype=np.float32)
    bb_np[:, 1] = np.repeat(np.asarray(b3, dtype=np.float32).reshape(2), 64)

    dve_h2 = not np.any(np.asarray(b2, dtype=np.float64) != 0.0)
    if dve_h2 not in _CACHE:
        _CACHE[dve_h2] = _build_nc(dve_h2)
    nc = _CACHE[dve_h2]

    in_maps = []
    for c in range(N_CORES):
        sl = slice(c * B_CORE, (c + 1) * B_CORE)
        in_maps.append({
            "xdr": np.ascontiguousarray(
                np.concatenate([w1_np, xdr[:, :, sl]], axis=2)),
            "w2": w2_np, "w3": w3_np, "bb": bb_np,
        })

    res = run_bass_kernel_spmd(nc, in_maps, core_ids=list(range(N_CORES)),
                               trace=TRACE)
    LAST["exec_time_ns"] = res.exec_time_ns
    LAST["results"] = res

    out = np.empty((B_TOTAL, 2), dtype=np.float32)
    b3f = np.asarray(b3, dtype=np.float64).reshape(1, 2)
    for c in range(N_CORES):
        o = res.results[c]["out"]        # [N_ROUNDS, 2, 64, OCOL] (z3)
        z3 = o.transpose(0, 2, 3, 1).reshape(B_CORE, 2).astype(np.float64)
        out[c * B_CORE:(c + 1) * B_CORE] = \
            (1.0 / (1.0 + np.exp(-(z3 + b3f)))).astype(np.float32)
    return out


# revision 69
# speedup vs baseline: 1.0002x; 1.0002x over previous
"""TRN2 Bass kernel for nn_HeartDisDet: embed-lookup + 44->256->128->2 MLP.

Strategy (8-way batch data-parallel, B=524288 -> 65536/core):
  - The 7 categorical embedding lookups are folded into the first matmul
    as exact polynomials in the raw category value (19 logical features;
    constant row carries b1). Host encodes the layout.
  - L1 runs as fp8e4m3 DoubleRow matmuls (0.5 cycles/row): the 19
    features and the folded weights are each split hi+lo into fp8 and
    laid out as K=50 rows (25 partitions x 2 k-tiles), making the fp8
    quantization error second-order (~1e-3) while halving PE cost.
  - h1 = tanh(z1) on ACT (bf16 out); L2 = W2^T h1 in bf16 (2 matmuls,
    K=256); L3 in bf16 with partition-packed PSUM (4 chunks/bank via
    tile_position), drained by DVE copies and repacked by DMAs.
  - h2 = tanh(z2): ~99/128 of chunks evaluate on the DVE via two
    PSUM-direct custom ops (1-Newton bitwise-seed reciprocal:
    OpA: yh ~ k0/(x^2+c); OpB: t = x(1 + a x^2) * yh; max |err| ~6e-3
    on |x|<=3.6), the rest on ACT with the bias fused.  The DVE path
    requires b2 == 0 (true for this problem's inputs); a nonzero b2
    falls back to all-ACT h2.  GPSIMD cannot read PSUM on real HW
    (and AP-scalars / 1-D Src1 in custom ops crash the device), so the
    Pool engine stays idle and the balance is ACT vs DVE.
  - Feature-major layout: activations are [hidden, samples] tiles.
    Emission is software-pipelined (L1/h1 | L2/h2 one chunk behind |
    L3/drain CDEPTH chunks behind) so no engine's in-order queue
    contains a same-chunk cross-engine round trip.
"""

import os
import re
import sys

if "/opt/trn_rl_repo" not in sys.path:
    sys.path.insert(0, "/opt/trn_rl_repo")
os.environ.setdefault("BASS_NEVER_TRACE", "1")

from contextlib import ExitStack

import ml_dtypes
import numpy as np

import concourse.bass as bass
import concourse.dve_ops as dvo
import concourse.mybir as mybir
import concourse.tile as tile
from concourse import bacc
from concourse.bass_utils import run_bass_kernel_spmd
from concourse.dve_spec import AluOp, Bin, One, Spec, Src0, Src1, C0, C1, C2, sq

N_CORES = 8
B_TOTAL = 524288
B_CORE = B_TOTAL // N_CORES   # 65536
CHUNK = 512                   # samples per matmul (PSUM bank = 512 fp32)
GROUP = 4096                  # samples per input DMA
N_GROUPS = B_CORE // GROUP    # 16
CHUNKS_PER_GROUP = GROUP // CHUNK    # 8
ROUND = 16384                 # samples per output round (32 chunks)
CHUNKS_PER_ROUND = ROUND // CHUNK    # 32
N_ROUNDS = B_CORE // ROUND    # 4
OCOL = ROUND // 64            # 256 output columns in the repack layout
K_P = 25                      # L1 DoubleRow contraction partitions
# k-tile rows: [19 features | 6 con_x-lo residual features] x {W_hi, W_lo}

# pipeline depth of the L3/drain stage behind the L1/h1 stage
CDEPTH = 4
# of every 128 chunks, this many h2 evaluations go to the DVE custom-op
# pair (PSUM-direct; requires b2 == 0); the rest stay on ACT.  GPSIMD
# cannot touch PSUM on real HW, so the Pool engine is unusable here and
# the balance is ACT (h1 + some h2) vs DVE (most h2).
NDVE128 = 101
# h2 chunks at the very end forced to ACT (tail convergence)
NTAIL2 = 5

TRACE = False
LAST = {}

# tanh(x) ~ x(1 + a s) * (k0/(s + c)), s = x^2, with the k0/(s+c) factor
# computed as a scaled 1-Newton reciprocal from a bitwise-NOT seed
# (constants tuned end-to-end in fp16/bf16, max err ~6.1e-3 on |x|<=3.6).
TA_A = 0.04404854372589698
TA_C = 2.844656450782092
TA_S1 = -0.3924570858306584
TA_M = 3.3526810765647515


def _register_op(op: "dvo.DveOp") -> "dvo.DveOp":
    """Register a custom DVE op, pinning uops_sha by trial-compiling (the
    sha mismatch error carries the computed value). Idempotent."""
    if op.name in dvo._SUB_OPCODE_FOR_NAME:
        return next(o for o in dvo.OPS if o.name == op.name)
    dvo._SUB_OPCODE_FOR_NAME[op.name] = dvo._CUSTOM_DVE_ROW_BASE + len(dvo.OPS)
    shas = {}
    for ver in ("v3", "v4"):
        try:
            op.compile(ver)
            shas[ver] = op.uops_sha.get(ver)
        except ValueError as e:
            m = re.search(rf"\({ver}: ([0-9a-f]+) ", str(e))
            if not m:
                raise
            shas[ver] = m.group(1)
    pinned = dvo.DveOp(name=op.name, spec=op.spec, subdim=op.subdim,
                       uops_sha=shas, perf_en=op.perf_en)
    dvo.OPS.append(pinned)
    dvo.CUSTOM_DVE_SPECS[pinned.name] = pinned.spec
    return pinned


def _ref_tanh_recip(in0, in1, c0, c1, c2):
    x = in0.astype(np.float32)
    d = x * x + np.float32(c0)
    nd = (~d.view(np.int32)).view(np.float32)
    y0 = nd * np.float32(c1)
    return y0 * (np.float32(c2) - d * y0)


_d = sq(Src0) + C0
_nd = Bin(AluOp.BITWISE_NOT, _d, _d)
_y0 = _nd * C1
TANH_RECIP_ANT = _register_op(dvo.DveOp(
    "TANH_RECIP_ANT",
    Spec(body=_y0 * (C2 - _d * _y0), reference=_ref_tanh_recip),
    subdim=False,
    uops_sha={},
))

TANH_NUM_ANT = _register_op(dvo.DveOp(
    "TANH_NUM_ANT",
    Spec(
        body=((sq(Src0) * C0 + One) * Src0) * Src1,
        reference=lambda in0, in1, c0, c1, c2: (
            (in0.astype(np.float32) * in0 * np.float32(c0) + np.float32(1.0))
            * in0 * in1).astype(np.float32),
    ),
    subdim=False,
    uops_sha={},
))


_CACHE = {}


def _h2_offload(c):
    """Evenly-spread Bresenham pattern: NDVE128 of every 128 h2 chunks -> DVE.
    The last NTAIL2 chunks stay on ACT so both engines drain together."""
    if c >= N_ROUNDS * CHUNKS_PER_ROUND - NTAIL2:
        return False
    cp = (c + 1) % 128
    return cp * NDVE128 // 128 != (cp + 1) * NDVE128 // 128


def _fold_weights(emb2, emb3, emb4, W1, b1):
    """Fold embedding tables + b1 into Wt [19, 256] (fp64 math)."""
    W1 = np.asarray(W1, dtype=np.float64)
    Wt = np.zeros((19, 256), dtype=np.float64)
    bt = np.asarray(b1, dtype=np.float64).copy()
    Wt[0:6] = W1[38:44]                          # con_x
    for i in range(3):                           # vocab-2: T0 + c*(T1-T0)
        T = np.asarray(emb2, np.float64)[i] @ W1[4 * i:4 * i + 4]
        Wt[6 + i] = T[1] - T[0]
        bt += T[0]
    for i in range(3):                           # vocab-3: quadratic fit
        S = np.asarray(emb3, np.float64)[i] @ W1[12 + 6 * i:18 + 6 * i]
        Wt[9 + i] = -1.5 * S[0] + 2.0 * S[1] - 0.5 * S[2]
        Wt[14 + i] = 0.5 * S[0] - S[1] + 0.5 * S[2]
        bt += S[0]
    # vocab-4 on basis {1, c, c^2, relu(c-2)}; exact on c in {0,1,2,3}
    S = np.asarray(emb4, np.float64) @ W1[30:38]
    V = np.array([[1, 0, 0, 0], [1, 1, 1, 0], [1, 2, 4, 0], [1, 3, 9, 1]],
                 dtype=np.float64)
    A = np.linalg.solve(V, S)
    Wt[12] = A[1]
    Wt[17] = A[2]
    Wt[18] = A[3]
    bt += A[0]
    Wt[13] = bt                                  # ones row carries bias
    return Wt


def _build_nc(dve_h2=True):
    f16 = mybir.dt.float16
    bf16 = mybir.dt.bfloat16
    f32 = mybir.dt.float32
    f8 = mybir.dt.float8e4
    DR = mybir.MatmulPerfMode.DoubleRow
    nc = bacc.Bacc(None, target_bir_lowering=False)
    # xdr carries w1 in its first 256 columns so one DMA lands both the
    # L1 weights and the first samples (cuts the fill latency chain)
    x_d = nc.dram_tensor("xdr", [K_P, 2, 256 + B_CORE], f8,
                         kind="ExternalInput")
    w2_d = nc.dram_tensor("w2", [2, 128, 128], bf16, kind="ExternalInput")
    w3_d = nc.dram_tensor("w3", [128, 2], bf16, kind="ExternalInput")
    bb_d = nc.dram_tensor("bb", [128, 2], f32, kind="ExternalInput")
    o_d = nc.dram_tensor("out", [N_ROUNDS, 2, 64, OCOL], f32,
                         kind="ExternalOutput")

    with nc.allow_low_precision(reason="fp8 DR L1, fp16 tanh, bf16 L2/L3"), \
            tile.TileContext(nc) as tc, ExitStack() as ctx:
        singles = ctx.enter_context(tc.tile_pool(name="singles", bufs=1))
        xpool = ctx.enter_context(tc.tile_pool(name="xg", bufs=4))
        h1pool = ctx.enter_context(tc.tile_pool(name="h1", bufs=5))
        h2pool = ctx.enter_context(tc.tile_pool(name="h2", bufs=8))
        rtpool = ctx.enter_context(tc.tile_pool(name="rt", bufs=3))
        opool = ctx.enter_context(tc.tile_pool(name="osb", bufs=2))
        stpool = ctx.enter_context(tc.tile_pool(name="stage", bufs=3))
        p1pool = ctx.enter_context(tc.tile_pool(name="p1", bufs=2, space="PSUM"))
        p2pool = ctx.enter_context(tc.tile_pool(name="p2", bufs=3, space="PSUM"))
        p3pool = ctx.enter_context(tc.tile_pool(name="p3", bufs=1, space="PSUM"))

        w2 = singles.tile([128, 2, 128], bf16)
        w3 = singles.tile([128, 2], bf16)
        bb = singles.tile([128, 2], f32)
        b2 = bb[:, 0:1]
        b3t = bb[:, 1:2]
        # tiny dummy matmul on a zeroed tile to start the PE p-state ramp
        # immediately, so the first real L1 matmuls run at full clock
        wsrc = singles.tile([8, 8], bf16)
        nc.gpsimd.memset(wsrc, 0.0)
        warm = p3pool.tile([8, 8], mybir.dt.float32, tag="p3")
        nc.tensor.matmul(warm, wsrc, wsrc, start=True, stop=True)

        tanh = mybir.ActivationFunctionType.Tanh
        sigm = mybir.ActivationFunctionType.Sigmoid


        n_chunks = N_ROUNDS * CHUNKS_PER_ROUND
        h1_of = {}
        h2_of = {}
        p3_of = {}
        osb_of = {}
        st4_of = {}

        # software-pipelined emission: per step s, stage A handles chunk s
        # (load/L1/h1 on ACT or the DVE custom-op pair), stage B chunk s-1
        # (L2 + ACT tanh h2), stage C chunk s-CDEPTH (L3/drain/output).
        for step in range(n_chunks + CDEPTH):
            # ---- stage A: chunk s ----
            if step < n_chunks:
                c = step
                g, cg = divmod(c, CHUNKS_PER_GROUP)
                if cg == 0:
                    if g == 0:
                        # group-0 tile leads with the 256 w1 columns
                        xg = xpool.tile([K_P, 2, 256 + GROUP], f8, tag="xg0")
                        w1 = xg[:, :, 0:256]
                        h = GROUP // 4
                        for q in range(4):
                            o0 = 256 + q * h if q else 0
                            o1 = 256 + (q + 1) * h
                            nc.sync.dma_start(
                                out=xg[:, :, o0:o1],
                                in_=x_d[:, :, o0:o1])
                            if q == 0:
                                nc.sync.dma_start(
                                    out=w2,
                                    in_=w2_d.rearrange("t p m -> p t m"))
                                nc.sync.dma_start(out=bb, in_=bb_d[:, :])
                        goff = 256
                    else:
                        xg = xpool.tile([K_P, 2, GROUP], f8, tag="xg")
                        nc.sync.dma_start(
                            out=xg,
                            in_=x_d[:, :, 256 + g * GROUP:256 + (g + 1) * GROUP])
                        goff = 0
                rhs = xg[:, :, goff + cg * CHUNK:goff + (cg + 1) * CHUNK]
                p1 = p1pool.tile([128, 2 * CHUNK], f32)
                nc.tensor.matmul(p1[:, 0:CHUNK], w1[:, :, 0:128], rhs,
                                 start=True, stop=True, perf_mode=DR)
                nc.tensor.matmul(p1[:, CHUNK:2 * CHUNK], w1[:, :, 128:256],
                                 rhs, start=True, stop=True, perf_mode=DR)
                h1 = h1pool.tile([128, 2 * CHUNK], bf16, tag="h1")
                nc.scalar.activation(h1, p1, tanh)
                h1_of[c] = h1
                if c == 0:
                    nc.sync.dma_start(out=w3, in_=w3_d[:, :])
            # ---- stage B: chunk s-1 ----
            if 0 <= step - 1 < n_chunks:
                c = step - 1
                h1 = h1_of.pop(c)
                p2 = p2pool.tile([128, CHUNK], f32)
                nc.tensor.matmul(p2, w2[:, 0, :], h1[:, 0:CHUNK],
                                 start=True, stop=False)
                nc.tensor.matmul(p2, w2[:, 1, :], h1[:, CHUNK:2 * CHUNK],
                                 start=False, stop=True)
                h2 = h2pool.tile([128, CHUNK], bf16, tag="h2")
                if dve_h2 and _h2_offload(c):
                    # PSUM-direct custom-op pair on the DVE (b2 == 0 here;
                    # AP scalars / 1-D Src1 crash the device, so a nonzero
                    # bias falls back to the ACT path below)
                    rt = rtpool.tile([128, CHUNK], f16, tag="rt")
                    nc.vector._custom_dve(TANH_RECIP_ANT, out=rt, in0=p2,
                                          s0=TA_C, s1=TA_S1, imm2=TA_M)
                    nc.vector._custom_dve(TANH_NUM_ANT, out=h2, in0=p2,
                                          in1=rt, s0=TA_A, s1=0.0)
                else:
                    nc.scalar.activation(h2, p2, tanh, bias=b2)
                h2_of[c] = h2
            # ---- stage C: chunk s-CDEPTH ----
            if 0 <= step - CDEPTH < n_chunks:
                c = step - CDEPTH
                rd, cc = divmod(c, CHUNKS_PER_ROUND)
                blk, i = divmod(cc, 4)           # z3 bank block / slot
                if i == 0:
                    p3_t = p3pool.tile([128, CHUNK], f32, tag="p3")
                    p3_of[blk] = p3_t
                p3 = p3_of[blk]
                h2v = h2_of.pop(c)
                # partition-packed L3: chunk slot i -> psum partitions 32i..+1
                nc.tensor.matmul(p3[32 * i:32 * i + 2, :], w3, h2v,
                                 start=True, stop=True,
                                 tile_position=(0, 32 * i))
                if i == 3:
                    # one DVE copy drains 4 chunks' [2, 512] z3 outputs
                    # (neither DMA nor GPSIMD can read PSUM), then the
                    # repack DMAs write z3 straight to DRAM; the host
                    # applies the final sigmoid in fp64.
                    stage = stpool.tile([128, CHUNK], f32, tag="stage")
                    nc.vector.tensor_copy(stage[0:98, :], p3[0:98, :])
                    # o_d[rd, o, 8b+2i+j, n] = z3(4b+i)[o, 256j+n]
                    for o in range(2):
                        nc.sync.dma_start(
                            out=o_d[rd, o, 8 * blk:8 * blk + 8, :],
                            in_=stage[o:o + 97:32, :]
                            .rearrange("p (j n) -> p j n", n=OCOL))
    nc.finalize()
    return nc


def kernel(con_x, cat_2, cat_3, cat_4, emb2, emb3, emb4,
           W1, b1, W2, b2, W3, b3):
    f8np = ml_dtypes.float8_e4m3
    bf16np = ml_dtypes.bfloat16
    B = con_x.shape[0]
    assert B == B_TOTAL

    Wt = _fold_weights(emb2, emb3, emb4, W1, b1)     # [19, 256] fp64

    # --- L1 DoubleRow operands: hi/lo fp8 split of features and weights ---
    # x rows (identical for both k-tiles): [19 features | 6 con_x lo-residual]
    xrow = np.empty((K_P, B), dtype=np.float32)
    cx = np.asarray(con_x, dtype=np.float32).T
    cxh = cx.astype(f8np).astype(np.float32)
    c3 = np.asarray(cat_3, dtype=np.float32).T
    c4 = np.asarray(cat_4, dtype=np.float32).T
    xrow[0:6] = cxh
    xrow[6:9] = np.asarray(cat_2, dtype=np.float32).T
    xrow[9:12] = c3
    xrow[12] = c4[0]
    xrow[13] = 1.0
    xrow[14:17] = c3 * c3
    xrow[17] = c4[0] * c4[0]
    xrow[18] = np.maximum(c4[0] - 2.0, 0.0)
    xrow[19:25] = cx - cxh                           # con_x lo residual
    x8 = xrow.astype(f8np)                           # [25, B]
    xdr = np.ascontiguousarray(
        np.broadcast_to(x8[:, None, :], (K_P, 2, B)))

    # weight k-tiles: hi rows, then lo rows (same x both times)
    Whi = Wt.astype(np.float32).astype(f8np).astype(np.float64)
    Wlo = (Wt - Whi).astype(np.float32).astype(f8np).astype(np.float64)
    # ktile0 = W_hi for all 25 x-rows, ktile1 = W_lo: together they form
    # (xh+xl)(Whi+Wlo) exactly, so the fp8 error is second-order.
    w1dr = np.zeros((K_P, 2, 256), dtype=np.float64)
    w1dr[0:19, 0] = Whi
    w1dr[19:25, 0] = Whi[0:6]
    w1dr[0:19, 1] = Wlo
    w1dr[19:25, 1] = Wlo[0:6]
    w1_np = w1dr.astype(np.float32).astype(f8np)          # [25, 2, 256]

    w2_np = np.ascontiguousarray(
        np.asarray(W2, dtype=np.float32).reshape(2, 128, 128).astype(bf16np))
    w3_np = np.ascontiguousarray(
        np.asarray(W3, dtype=np.float32).astype(bf16np))
    bb_np = np.empty((128, 2), dtype=np.float32)
    bb_np[:, 0] = np.asarray(b2, dtype=np.float32)
    bb_np[:, 1] = np.repeat(np.asarray(b3, dtype=np.float32).reshape(2), 64)

    dve_h2 = not np.any(np.asarray(b2, dtype=np.float64) != 0.0)
    if dve_h2 not in _CACHE:
        _CACHE[dve_h2] = _build_nc(dve_h2)
    nc = _CACHE[dve_h2]

    in_maps = []
    for c in range(N_CORES):
        sl = slice(c * B_CORE, (c + 1) * B_CORE)
        in_maps.append({
            "xdr": np.ascontiguousarray(
                np.concatenate([w1_np, xdr[:, :, sl]], axis=2)),
            "w2": w2_np, "w3": w3_np, "bb": bb_np,
        })

    res = run_bass_kernel_spmd(nc, in_maps, core_ids=list(range(N_CORES)),
                               trace=TRACE)
    LAST["exec_time_ns"] = res.exec_time_ns
    LAST["results"] = res

    out = np.empty((B_TOTAL, 2), dtype=np.float32)
    b3f = np.asarray(b3, dtype=np.float64).reshape(1, 2)
    for c in range(N_CORES):
        o = res.results[c]["out"]        # [N_ROUNDS, 2, 64, OCOL] (z3)
        z3 = o.transpose(0, 2, 3, 1).reshape(B_CORE, 2).astype(np.float64)
        out[c * B_CORE:(c + 1) * B_CORE] = \
            (1.0 / (1.0 + np.exp(-(z3 + b3f)))).astype(np.float32)
    return out
